# revision 12
# baseline (speedup 1.0000x reference)
# Trainium2 Bass kernel for nn_Attention_80779744903426
#
# Reference computation (b=4, n=2048, c=1024, h=16, d=64):
#   qkv = x @ w_qkv ; split to q,k,v per head
#   attn = softmax(q k^T / sqrt(c)) ; out = (attn v) concat ; y = out @ w_proj + b_proj
#
# Sharding (8 cores): data-parallel over batch (4) x tensor-parallel over
# head-groups (2 groups of 8 heads, Megatron-style). Each core computes a
# partial y for its batch from its 8 heads; host sums the two partials per
# batch and adds b_proj.
#
# Per-core program (all matmuls bf16, fp32 PSUM accumulation):
#   Iteration order is PAIR-major (pair outer, q-chunk inner) so each head
#   pair's K^T is needed a full 4 iterations after the previous one --
#   projection chains spread evenly as fillers instead of bunching at the
#   start.
#   A) minimal preamble: K^T chains for pair 0 + Q^T(pair0, chunk0) only.
#      V = x @ wv and all later Q^T/K^T chains are interleaved into the
#      attention iterations as filler thunks, paced just-in-time ahead of
#      their consumers.
#   B) per (pair, q-chunk): 32 S slots (slot 2k+hh = S^T k-tile k of head
#      hh). The two heads sit on disjoint PE row groups (stationary K=64 at
#      base partition 0 / 64), so the slot pair streams CONCURRENTLY
#      through the array. exp via ACT over 3-slot PSUM batches (softmax
#      scale folded in), bf16 out into a ring of [128,3,512] P~ tiles
#      (bufs=14) -- the ring lets the previous iteration's PV free slots as
#      the current iteration produces them. O'[65,q] = [V_h | 1]^T @ P~^T
#      (ones column = fused softmax denominator); PV of the previous
#      (pair,chunk) is interleaved between S batches of the current one,
#      consuming P~ ring tiles in slot order. Normalization: copy O' out of
#      PSUM, fast-reciprocal of the sums row, partition-broadcast via a
#      DRAM-bounce DMA on the gpsimd queue, multiply into O^T.
#   C) y = O^T(stationary) @ wp(moving) over 4 o-tiles; pair-major defers
#      proj(qc) until pair 3's chunk qc is normalized (iters 14, 15, tail).

import numpy as np

DIM = 1024
N = 2048
B = 4
NH = 16
HD = 64
SCALE = 1.0 / DIM**0.5

HPC = 8            # heads per core
PAIRS = HPC // 2   # head pairs (row-tiled together)
CT = 8             # contraction tiles over c=1024
NT = 16            # n tiles of 128
ACH = 512          # phase-A n-chunk
QCH = 512          # phase-B q-chunk
NQC = N // QCH     # 4 q-chunks
KT = 16            # k tiles of 128 in attention
NSLOT = 2 * KT     # S slots per (pair, chunk): slot = 2k + hh

# 11 ACT batches per iteration: 10x3 + 1x2 slots; P~ ring tile j of an
# iteration holds slots [3j, 3j+3). PV segment j of the previous iteration
# consumes exactly ring tile j.
SLOT_BATCHES = [(3 * i, 3) for i in range(10)] + [(30, 2)]
NBATCH = len(SLOT_BATCHES)

_CACHE = {}


def _build_nc():
    import concourse.bass as bass
    from concourse import bacc, mybir, tile

    f32 = mybir.dt.float32
    bf16 = mybir.dt.bfloat16
    EXP = mybir.ActivationFunctionType.Exp

    nc = bacc.Bacc("TRN2", target_bir_lowering=False, debug=False)

    xT_d = nc.dram_tensor("xT", [DIM, N], bf16, kind="ExternalInput").ap()
    wqk_d = nc.dram_tensor("wqk", [DIM, 1024], bf16, kind="ExternalInput").ap()
    wv_d = nc.dram_tensor("wv", [DIM, 512], bf16, kind="ExternalInput").ap()
    wp_d = nc.dram_tensor("wp", [512, DIM], bf16, kind="ExternalInput").ap()
    y_d = nc.dram_tensor("y", [N, DIM], f32, kind="ExternalOutput").ap()

    with tile.TileContext(nc) as tc:
        with (
            tc.tile_pool(name="p16", bufs=5) as p16,      # xt staging
            tc.tile_pool(name="pt", bufs=14) as ptp,      # P~ ring tiles
            tc.tile_pool(name="wqk", bufs=1) as wqkp,
            tc.tile_pool(name="wv", bufs=1) as wvp,
            tc.tile_pool(name="wp", bufs=1) as wpp,
            tc.tile_pool(name="v", bufs=1) as vp,
            tc.tile_pool(name="ot", bufs=1) as otp,
            tc.tile_pool(name="misc", bufs=2) as miscp,
            tc.tile_pool(name="ps", bufs=1, space="PSUM") as psp,
            tc.tile_pool(name="dram", bufs=1, space="DRAM") as dp,
        ):
            # ---- static tiles ----
            wqk_sb = wqkp.tile([128, CT, 1024], bf16)
            for ct in range(CT):
                nc.sync.dma_start(wqk_sb[:, ct, :], wqk_d[128 * ct : 128 * (ct + 1), :])
            wv_sb = wvp.tile([128, CT, 512], bf16)
            for ct in range(CT):
                nc.sync.dma_start(wv_sb[:, ct, :], wv_d[128 * ct : 128 * (ct + 1), :])
            wp_sb = wpp.tile([128, 4, 1024], bf16)
            for ot in range(4):
                nc.sync.dma_start(wp_sb[:, ot, :], wp_d[128 * ot : 128 * (ot + 1), :])

            v_sb = vp.tile([128, NT, HPC, HD + 1], bf16)  # [k-part, k-tile, head, d | 1]
            nc.vector.memset(v_sb[:, :, :, HD], 1.0)

            ot_sb = otp.tile([128, PAIRS, N], bf16)  # O^T rows: pair p = rows 128p..
            # Q^T/K^T 2-slot rings: pair p lives in slot p % 2
            qt_all = otp.tile([128, 2, N], bf16, name="qt_all")
            kt_all = otp.tile([128, 2, N], bf16, name="kt_all")

            xT_r = xT_d.rearrange("(t p) n -> p t n", p=128)

            # ---- phase A helpers ----
            def emit_chain(xt, mt, ach, dst):
                # one 128-row output tile of x @ wqk (mt<4: Q^T pair mt,
                # mt>=4: K^T pair mt-4), n-chunk ach
                qps = psp.tile([128, 512], f32, tag="acc", bufs=2, name="qps")
                for ct in range(CT):
                    nc.tensor.matmul(
                        qps, wqk_sb[:, ct, 128 * mt : 128 * (mt + 1)],
                        xt[:, ct, :], start=(ct == 0), stop=(ct == CT - 1))
                nc.vector.tensor_copy(
                    dst[:, ACH * ach : ACH * (ach + 1)], qps)

            def load_xt(ach):
                xt = p16.tile([128, CT, ACH], bf16, tag="big16", name="xt")
                nc.sync.dma_start(xt, xT_r[:, :, ACH * ach : ACH * (ach + 1)])
                return xt

            def emit_v_group(xt, nt):
                # V rows for n-tile nt (all 8 heads) from resident xt of
                # chunk nt // 4
                sub = nt % 4
                vps = psp.tile([128, 512], f32, tag="acc", bufs=2, name="vps")
                for ct in range(CT):
                    nc.tensor.matmul(vps, xt[:, ct, 128 * sub : 128 * (sub + 1)],
                                     wv_sb[:, ct, :], start=(ct == 0),
                                     stop=(ct == CT - 1))
                nc.vector.tensor_copy(
                    v_sb[:, nt, :, 0:HD],
                    vps.rearrange("p (h d) -> p h d", h=HPC),
                )

            # ---- minimal preamble: K^T pair0 (4 chunks) + Q^T(p0, qc0) ----
            p1_xts = [load_xt(ach) for ach in range(N // ACH)]
            for a in range(4):
                emit_chain(p1_xts[a], 4, a, kt_all[:, 0, :])
            emit_chain(p1_xts[0], 0, 0, qt_all[:, 0, :])

            # ---- filler thunks ----
            def v_thunk(ach, gs):
                def t():
                    for g in gs:
                        emit_v_group(p1_xts[ach], 4 * ach + g)
                return t

            def qt_res_thunk(p, qc):
                # Q^T(pair p, chunk qc) from a resident preamble xt
                def t():
                    emit_chain(p1_xts[qc], p, qc, qt_all[:, p % 2, :])
                return t

            def kt_thunk(p, ach):
                def t():
                    xt = load_xt(ach)
                    emit_chain(xt, 4 + p, ach, kt_all[:, p % 2, :])
                return t

            def qt_thunk(p, qc):
                def t():
                    xt = load_xt(qc)
                    emit_chain(xt, p, qc, qt_all[:, p % 2, :])
                return t

            def emit_pv_segment(st, seg):
                p0, ptl, opsl = st
                s0, sn = SLOT_BATCHES[seg]
                for i in range(sn):
                    s = s0 + i
                    k, hh = s // 2, s % 2
                    nc.tensor.matmul(opsl[hh], v_sb[:, k, 2 * p0 + hh, :],
                                     ptl[seg][:, i, :],
                                     start=(k == 0), stop=(k == KT - 1))

            def emit_norm(st, qc0):
                # Copy O' out of PSUM first so the PSUM slot recycles without
                # waiting for the reciprocal/broadcast chain. Bounce DMAs ride
                # the gpsimd SWDGE queue so they never head-of-line-block the
                # sync queue carrying bulk loads.
                p0, ptl, opsl = st
                for hh in range(2):
                    ops = opsl[hh]
                    ostg = miscp.tile([HD, QCH], f32, tag="ostg", bufs=3,
                                      name="ostg")
                    nc.vector.tensor_copy(ostg, ops[0:HD, :])
                    # denominator row staged to partition 0: the custom-DVE
                    # reciprocal_approx_fast misreads non-zero base partitions
                    den = miscp.tile([1, QCH], f32, tag="den", bufs=2, name="den")
                    nc.vector.tensor_copy(den, ops[HD : HD + 1, :])
                    rcp = miscp.tile([1, QCH], f32, tag="rcp", bufs=2, name="rcp")
                    nc.vector.reciprocal_approx_fast(rcp, den)
                    rcp_d = dp.tile([1, QCH], f32, tag="rcpd", bufs=4, name="rcpd")
                    nc.gpsimd.dma_start(rcp_d, rcp)
                    bc = miscp.tile([64, QCH], f32, tag="bc", bufs=3, name="bc")
                    rap = rcp_d[:]
                    nc.gpsimd.dma_start(
                        bc,
                        bass.AP(tensor=rap.tensor, offset=rap.offset,
                                ap=[[0, 64]] + list(rap.ap[1:])),
                    )
                    nc.vector.tensor_mul(
                        ot_sb[64 * hh : 64 * (hh + 1), p0, QCH * qc0 : QCH * (qc0 + 1)],
                        ostg,
                        bc,
                    )

            def emit_proj_half(qc0, half):
                # y columns for q-chunk qc0 (needs ot_sb[:, :, chunk] complete)
                nt = (QCH // 128) * qc0 + half * 2
                for nt2 in (nt, nt + 1):
                    for yc in range(2):
                        yps = psp.tile([128, 512], f32, tag="acc", bufs=2, name="yps")
                        for ot in range(4):
                            nc.tensor.matmul(
                                yps, ot_sb[:, ot, 128 * nt2 : 128 * (nt2 + 1)],
                                wp_sb[:, ot, 512 * yc : 512 * (yc + 1)],
                                start=(ot == 0), stop=(ot == 3))
                        stg = miscp.tile([128, 512], f32, tag="ystg", bufs=2,
                                         name="ystg")
                        nc.vector.tensor_copy(stg, yps)
                        nc.sync.dma_start(
                            y_d[128 * nt2 : 128 * (nt2 + 1), 512 * yc : 512 * (yc + 1)],
                            stg,
                        )

            def proj_thunk(qc0, half):
                return lambda: emit_proj_half(qc0, half)

            # filler schedule, iter = 4*p + qc (pair-major).
            # deadlines: K^T pair p fully by iter 4p (k-tile k by batch 2k/3);
            # Q^T(p, qc) by iter 4p+qc; V n-tile nt by iter-1 batch ~2nt/3;
            # proj(qc) after norm(p3, qc) which lands at end of iter 13+qc.
            # ALL V chains must be iter-0 fillers: in any later iteration the
            # acc-tag PSUM ring makes new chain tiles wait behind the live PV
            # accumulators (opsl), whose release needs the PV that consumes
            # V -- a cycle. Iter 0 has no live opsl, so V runs promptly.
            EXTRAS = {
                0: [v_thunk(0, [0, 1]), v_thunk(0, [2, 3]), v_thunk(1, [0, 1]),
                    v_thunk(1, [2, 3]), v_thunk(2, [0, 1]), v_thunk(2, [2, 3]),
                    v_thunk(3, [0, 1]), v_thunk(3, [2, 3]), qt_res_thunk(0, 1)],
                1: [qt_res_thunk(0, 2)],
                2: [qt_res_thunk(0, 3), kt_thunk(1, 0), kt_thunk(1, 1)],
                3: [kt_thunk(1, 2), kt_thunk(1, 3), qt_thunk(1, 0)],
                4: [qt_thunk(1, 1)],
                5: [qt_thunk(1, 2)],
                6: [qt_thunk(1, 3), kt_thunk(2, 0), kt_thunk(2, 1)],
                7: [kt_thunk(2, 2), kt_thunk(2, 3), qt_thunk(2, 0)],
                8: [qt_thunk(2, 1)],
                9: [qt_thunk(2, 2)],
                10: [qt_thunk(2, 3), kt_thunk(3, 0), kt_thunk(3, 1)],
                11: [kt_thunk(3, 2), kt_thunk(3, 3), qt_thunk(3, 0)],
                12: [qt_thunk(3, 1)],
                13: [qt_thunk(3, 2)],
                14: [qt_thunk(3, 3), proj_thunk(0, 0), proj_thunk(0, 1)],
                15: [proj_thunk(1, 0), proj_thunk(1, 1)],
            }

            pv_st = None
            pv_qc = None
            it = -1
            for p in range(PAIRS):
                for qc in range(NQC):
                    it += 1
                    kt_sb = kt_all[:, p % 2, :]
                    qt_sb = qt_all[:, p % 2, QCH * qc : QCH * (qc + 1)]
                    extras = EXTRAS.get(it, [])
                    ptl = []
                    for bi, (s0, sn) in enumerate(SLOT_BATCHES):
                        sps = psp.tile([128, sn, QCH], f32, tag="sb3", bufs=2,
                                       name="sps")
                        for i in range(sn):
                            s = s0 + i
                            k, hh = s // 2, s % 2
                            sl = slice(64 * hh, 64 * (hh + 1))
                            nc.tensor.matmul(
                                sps[:, i, :],
                                kt_sb[sl, 128 * k : 128 * (k + 1)],
                                qt_sb[sl, :], start=True, stop=True)
                        ptile = ptp.tile([128, 3, QCH], bf16, name="pt")
                        ptl.append(ptile)
                        nc.scalar.activation(
                            out=ptile[:, 0:sn, :],
                            in_=sps,
                            func=EXP,
                            scale=float(SCALE),
                        )
                        if pv_st is not None:
                            emit_pv_segment(pv_st, bi)
                        if bi < len(extras):
                            extras[bi]()
                    if pv_st is not None:
                        emit_norm(pv_st, pv_qc)
                    opsl = [
                        psp.tile([HD + 1, QCH], f32, tag="acc", bufs=2,
                                 name=f"ops{hh}")
                        for hh in range(2)
                    ]
                    pv_st = (p, ptl, opsl)
                    pv_qc = qc
            # drain the last (p3, qc3)
            for seg in range(NBATCH):
                emit_pv_segment(pv_st, seg)
            emit_norm(pv_st, pv_qc)
            emit_proj_half(2, 0)
            emit_proj_half(2, 1)
            emit_proj_half(3, 0)
            emit_proj_half(3, 1)

    nc.compile()
    return nc


def get_nc():
    if "nc" not in _CACHE:
        _CACHE["nc"] = _build_nc()
    return _CACHE["nc"]


def make_in_maps(x, w_qkv, w_proj):
    import ml_dtypes

    bf = ml_dtypes.bfloat16
    in_maps = []
    for c in range(8):
        b, g = c // 2, c % 2
        in_maps.append({
            "xT": np.ascontiguousarray(x[b].T).astype(bf),
            "wqk": np.ascontiguousarray(
                np.concatenate(
                    [w_qkv[:, 512 * g : 512 * (g + 1)],
                     w_qkv[:, 1024 + 512 * g : 1024 + 512 * (g + 1)]], axis=1
                )).astype(bf),
            "wv": np.ascontiguousarray(
                w_qkv[:, 2048 + 512 * g : 2048 + 512 * (g + 1)]).astype(bf),
            "wp": np.ascontiguousarray(
                w_proj[512 * g : 512 * (g + 1), :]).astype(bf),
        })
    return in_maps


def kernel(x, w_qkv, w_proj, b_proj):
    from concourse.bass_utils import run_bass_kernel_spmd

    x = np.asarray(x, dtype=np.float32)
    w_qkv = np.asarray(w_qkv, dtype=np.float32)
    w_proj = np.asarray(w_proj, dtype=np.float32)
    b_proj = np.asarray(b_proj, dtype=np.float32)

    nc = get_nc()
    in_maps = make_in_maps(x, w_qkv, w_proj)
    res = run_bass_kernel_spmd(nc, in_maps, list(range(8))).results

    out = np.zeros((B, N, DIM), dtype=np.float32)
    for c in range(8):
        out[c // 2] += res[c]["y"]
    return out + b_proj


# revision 26
# speedup vs baseline: 1.0623x; 1.0623x over previous
# Trainium2 Bass kernel for nn_Attention_80779744903426
#
# Reference computation (b=4, n=2048, c=1024, h=16, d=64):
#   qkv = x @ w_qkv ; split to q,k,v per head
#   attn = softmax(q k^T / sqrt(c)) ; out = (attn v) concat ; y = out @ w_proj + b_proj
#
# Sharding (8 cores): data-parallel over batch (4) x tensor-parallel over
# head-groups (2 groups of 8 heads, Megatron-style). Each core computes a
# partial y for its batch from its 8 heads; host sums the two partials per
# batch and adds b_proj.
#
# Per-core program (all matmuls bf16, fp32 PSUM accumulation):
#   Iteration order is PAIR-major (pair outer, q-chunk inner) so each head
#   pair's K^T is needed a full 4 iterations after the previous one --
#   projection chains spread evenly as fillers instead of bunching at the
#   start.
#   A) minimal preamble: K^T chains for pair 0 + Q^T(pair0, chunk0) only.
#      V = x @ wv and all later Q^T/K^T chains are interleaved into the
#      attention iterations as filler thunks, paced just-in-time ahead of
#      their consumers.
#   B) per (pair, q-chunk): 32 S slots (slot 2k+hh = S^T k-tile k of head
#      hh). The two heads sit on disjoint PE row groups (stationary K=64 at
#      base partition 0 / 64), so the slot pair streams CONCURRENTLY
#      through the array. exp via ACT over 3-slot PSUM batches (softmax
#      scale folded in), bf16 out into a ring of [128,3,512] P~ tiles
#      (bufs=14) -- the ring lets the previous iteration's PV free slots as
#      the current iteration produces them. O'[65,q] = [V_h | 1]^T @ P~^T
#      (ones column = fused softmax denominator); PV of the previous
#      (pair,chunk) is interleaved between S batches of the current one,
#      consuming P~ ring tiles in slot order. Normalization: copy O' out of
#      PSUM, fast-reciprocal of the sums row, partition-broadcast via a
#      DRAM-bounce DMA on the gpsimd queue, multiply into O^T.
#   C) y = O^T(stationary) @ wp(moving) over 4 o-tiles; pair-major defers
#      proj(qc) until pair 3's chunk qc is normalized (iters 14, 15, tail).

import numpy as np

DIM = 1024
N = 2048
B = 4
NH = 16
HD = 64
SCALE = 1.0 / DIM**0.5

HPC = 8            # heads per core
PAIRS = HPC // 2   # head pairs (row-tiled together)
CT = 8             # contraction tiles over c=1024
NT = 16            # n tiles of 128
ACH = 512          # phase-A n-chunk
QCH = 512          # phase-B q-chunk
NQC = N // QCH     # 4 q-chunks
KT = 16            # k tiles of 128 in attention
NSLOT = 2 * KT     # S slots per (pair, chunk): slot = 2k + hh

# 13 ACT batches per iteration, sizes alternating 3,2 (PSUM tags sbA/sbB --
# 5 banks total, leaving banks for the PV accumulators and one transient
# chain bank). PV segment j of the previous iteration consumes ring tile j.
_sizes = [3, 2] * 6 + [2]
SLOT_BATCHES = []
_s0 = 0
for _n in _sizes:
    SLOT_BATCHES.append((_s0, _n))
    _s0 += _n
assert _s0 == NSLOT
NBATCH = len(SLOT_BATCHES)

_CACHE = {}


def _build_nc():
    import concourse.bass as bass
    from concourse import bacc, mybir, tile

    f32 = mybir.dt.float32
    bf16 = mybir.dt.bfloat16
    EXP = mybir.ActivationFunctionType.Exp

    nc = bacc.Bacc("TRN2", target_bir_lowering=False, debug=False)

    xT_d = nc.dram_tensor("xT", [DIM, N], bf16, kind="ExternalInput").ap()
    wqk_d = nc.dram_tensor("wqk", [DIM, 1024], bf16, kind="ExternalInput").ap()
    wv_d = nc.dram_tensor("wv", [DIM, 512], bf16, kind="ExternalInput").ap()
    wp_d = nc.dram_tensor("wp", [512, DIM], bf16, kind="ExternalInput").ap()
    y_d = nc.dram_tensor("y", [N, DIM], f32, kind="ExternalOutput").ap()

    with tile.TileContext(nc) as tc:
        with (
            tc.tile_pool(name="p16", bufs=5) as p16,      # xt staging
            tc.tile_pool(name="pt", bufs=16) as ptp,      # P~ ring tiles
            tc.tile_pool(name="wqk", bufs=1) as wqkp,
            tc.tile_pool(name="wv", bufs=1) as wvp,
            tc.tile_pool(name="wp", bufs=1) as wpp,
            tc.tile_pool(name="v", bufs=1) as vp,
            tc.tile_pool(name="ot", bufs=1) as otp,
            tc.tile_pool(name="misc", bufs=2) as miscp,
            tc.tile_pool(name="ps", bufs=1, space="PSUM") as psp,
            tc.tile_pool(name="dram", bufs=1, space="DRAM") as dp,
        ):
            # ---- static tiles ----
            # DMA order matters at the front: the first S batch needs
            # xt(chunk0) + the K^T columns of wqk; wp is not needed until
            # proj (iter 14) and wv not until the iter-0 V fillers.
            xT_r0 = xT_d.rearrange("(t p) n -> p t n", p=128)
            p1_xts = []
            for ach in range(N // ACH):
                xt = p16.tile([128, CT, ACH], bf16, tag="big16", name="xt")
                nc.sync.dma_start(xt, xT_r0[:, :, ACH * ach : ACH * (ach + 1)])
                p1_xts.append(xt)
            wqk_sb = wqkp.tile([128, CT, 1024], bf16)
            for ct in range(CT):
                nc.sync.dma_start(wqk_sb[:, ct, 512:1024],
                                  wqk_d[128 * ct : 128 * (ct + 1), 512:1024])
            for ct in range(CT):
                nc.sync.dma_start(wqk_sb[:, ct, 0:512],
                                  wqk_d[128 * ct : 128 * (ct + 1), 0:512])
            wv_sb = wvp.tile([128, CT, 512], bf16)
            for ct in range(CT):
                nc.sync.dma_start(wv_sb[:, ct, :], wv_d[128 * ct : 128 * (ct + 1), :])
            wp_sb = wpp.tile([128, 4, 1024], bf16)
            for ot in range(4):
                nc.sync.dma_start(wp_sb[:, ot, :], wp_d[128 * ot : 128 * (ot + 1), :])

            v_sb = vp.tile([128, NT, HPC, HD + 1], bf16)  # [k-part, k-tile, head, d | 1]
            nc.vector.memset(v_sb[:, :, :, HD], 1.0)

            ot_sb = otp.tile([128, PAIRS, N], bf16)  # O^T rows: pair p = rows 128p..
            # Q^T/K^T 2-slot rings: pair p lives in slot p % 2
            qt_all = otp.tile([128, 2, N], bf16, name="qt_all")
            kt_all = otp.tile([128, 2, N], bf16, name="kt_all")

            xT_r = xT_d.rearrange("(t p) n -> p t n", p=128)

            # ---- phase A helpers ----
            # transient chain accumulators live on their own "tr" bank (or
            # "ops" while it is still free in iter 0) so chains NEVER queue
            # behind the live PV accumulators -- the v3 deadlock/burst.
            def ps_tmp(tag):
                return psp.tile([128, 512], f32, tag=tag,
                                bufs=(2 if tag == "ops" else 1), name="c" + tag)

            def emit_chain(xt, mt, ach, dst, tag="tr"):
                # one 128-row output tile of x @ wqk (mt<4: Q^T pair mt,
                # mt>=4: K^T pair mt-4), n-chunk ach
                qps = ps_tmp(tag)
                for ct in range(CT):
                    nc.tensor.matmul(
                        qps, wqk_sb[:, ct, 128 * mt : 128 * (mt + 1)],
                        xt[:, ct, :], start=(ct == 0), stop=(ct == CT - 1))
                nc.vector.tensor_copy(
                    dst[:, ACH * ach : ACH * (ach + 1)], qps)

            def load_xt(ach):
                xt = p16.tile([128, CT, ACH], bf16, tag="big16", name="xt")
                nc.sync.dma_start(xt, xT_r[:, :, ACH * ach : ACH * (ach + 1)])
                return xt

            def emit_v_group(xt, nt, tag="tr"):
                # V rows for n-tile nt (all 8 heads) from resident xt of
                # chunk nt // 4
                sub = nt % 4
                vps = ps_tmp(tag)
                for ct in range(CT):
                    nc.tensor.matmul(vps, xt[:, ct, 128 * sub : 128 * (sub + 1)],
                                     wv_sb[:, ct, :], start=(ct == 0),
                                     stop=(ct == CT - 1))
                nc.vector.tensor_copy(
                    v_sb[:, nt, :, 0:HD],
                    vps.rearrange("p (h d) -> p h d", h=HPC),
                )

            # ---- minimal preamble: K^T pair0 (4 chunks) + Q^T(p0, qc0) ----
            for a in range(4):
                emit_chain(p1_xts[a], 4, a, kt_all[:, 0, :],
                           tag=("ops" if a % 2 else "tr"))
            emit_chain(p1_xts[0], 0, 0, qt_all[:, 0, :], tag="ops")

            # ---- filler thunks ----
            def v_thunk(ach, gs, tag="tr"):
                def t():
                    for g in gs:
                        emit_v_group(p1_xts[ach], 4 * ach + g, tag=tag)
                return t

            def qt_res_thunk(p, qc):
                # Q^T(pair p, chunk qc) from a resident preamble xt
                def t():
                    emit_chain(p1_xts[qc], p, qc, qt_all[:, p % 2, :])
                return t

            def kt_thunk(p, ach):
                def t():
                    xt = load_xt(ach)
                    emit_chain(xt, 4 + p, ach, kt_all[:, p % 2, :])
                return t

            def qt_thunk(p, qc):
                def t():
                    xt = load_xt(qc)
                    emit_chain(xt, p, qc, qt_all[:, p % 2, :])
                return t

            def emit_pv_segment(st, seg):
                p0, ptl, opsl = st
                s0, sn = SLOT_BATCHES[seg]
                for i in range(sn):
                    s = s0 + i
                    k, hh = s // 2, s % 2
                    nc.tensor.matmul(opsl[hh], v_sb[:, k, 2 * p0 + hh, :],
                                     ptl[seg][:, i, :],
                                     start=(k == 0), stop=(k == KT - 1))

            def emit_norm(st, qc0):
                # Copy O' out of PSUM first so the PSUM slot recycles without
                # waiting for the reciprocal/broadcast chain. Bounce DMAs ride
                # the gpsimd SWDGE queue so they never head-of-line-block the
                # sync queue carrying bulk loads.
                p0, ptl, opsl = st
                for hh in range(2):
                    ops = opsl[hh]
                    ostg = miscp.tile([HD, QCH], f32, tag="ostg", bufs=3,
                                      name="ostg")
                    nc.vector.tensor_copy(ostg, ops[0:HD, :])
                    # denominator row staged to partition 0: the custom-DVE
                    # reciprocal_approx_fast misreads non-zero base partitions
                    den = miscp.tile([1, QCH], f32, tag="den", bufs=2, name="den")
                    nc.vector.tensor_copy(den, ops[HD : HD + 1, :])
                    rcp = miscp.tile([1, QCH], f32, tag="rcp", bufs=2, name="rcp")
                    nc.vector.reciprocal_approx_fast(rcp, den)
                    rcp_d = dp.tile([1, QCH], f32, tag="rcpd", bufs=4, name="rcpd")
                    nc.gpsimd.dma_start(rcp_d, rcp)
                    bc = miscp.tile([64, QCH], f32, tag="bc", bufs=3, name="bc")
                    rap = rcp_d[:]
                    nc.gpsimd.dma_start(
                        bc,
                        bass.AP(tensor=rap.tensor, offset=rap.offset,
                                ap=[[0, 64]] + list(rap.ap[1:])),
                    )
                    nc.vector.tensor_mul(
                        ot_sb[64 * hh : 64 * (hh + 1), p0, QCH * qc0 : QCH * (qc0 + 1)],
                        ostg,
                        bc,
                    )

            def emit_proj_half(qc0, half, tags=("tr", "tr")):
                # y columns for q-chunk qc0 (needs ot_sb[:, :, chunk] complete)
                nt = (QCH // 128) * qc0 + half * 2
                for nt2 in (nt, nt + 1):
                    for yc in range(2):
                        tg = tags[(nt2 + yc) % 2]
                        yps = (psp.tile([128, 512], f32, tag=tg, bufs=1,
                                        name="yps")
                               if tg in ("sbA", "sbB")
                               else ps_tmp(tg))
                        for ot in range(4):
                            nc.tensor.matmul(
                                yps, ot_sb[:, ot, 128 * nt2 : 128 * (nt2 + 1)],
                                wp_sb[:, ot, 512 * yc : 512 * (yc + 1)],
                                start=(ot == 0), stop=(ot == 3))
                        stg = miscp.tile([128, 512], f32, tag="ystg", bufs=2,
                                         name="ystg")
                        nc.vector.tensor_copy(stg, yps)
                        nc.sync.dma_start(
                            y_d[128 * nt2 : 128 * (nt2 + 1), 512 * yc : 512 * (yc + 1)],
                            stg,
                        )

            def proj_thunk(qc0, half):
                return lambda: emit_proj_half(qc0, half)

            # filler schedule, iter = 4*p + qc (pair-major).
            # deadlines: K^T pair p fully by iter 4p (k-tile k by batch 2k/3);
            # Q^T(p, qc) by iter 4p+qc; V n-tile nt by iter-1 batch ~2nt/3;
            # proj(qc) after norm(p3, qc) which lands at end of iter 13+qc.
            # ALL V chains are iter-0 fillers: PV of iter 0 (running in
            # iter 1) consumes every V n-tile, and iter 0 is the only window
            # where the "ops" banks are still free to double chain
            # throughput (alternating ops/tr).
            EXTRAS = {
                0: [v_thunk(0, [0, 1], "ops"), v_thunk(0, [2, 3]),
                    v_thunk(1, [0, 1], "ops"), v_thunk(1, [2, 3]),
                    v_thunk(2, [0, 1], "ops"), v_thunk(2, [2, 3]),
                    v_thunk(3, [0, 1], "ops"), v_thunk(3, [2, 3]),
                    qt_res_thunk(0, 1)],
                1: [qt_res_thunk(0, 2)],
                2: [qt_res_thunk(0, 3), kt_thunk(1, 0), kt_thunk(1, 1)],
                3: [kt_thunk(1, 2), kt_thunk(1, 3), qt_thunk(1, 0)],
                4: [qt_thunk(1, 1)],
                5: [qt_thunk(1, 2)],
                6: [qt_thunk(1, 3), kt_thunk(2, 0), kt_thunk(2, 1)],
                7: [kt_thunk(2, 2), kt_thunk(2, 3), qt_thunk(2, 0)],
                8: [qt_thunk(2, 1)],
                9: [qt_thunk(2, 2)],
                10: [qt_thunk(2, 3), kt_thunk(3, 0), kt_thunk(3, 1)],
                11: [kt_thunk(3, 2), kt_thunk(3, 3), qt_thunk(3, 0)],
                12: [qt_thunk(3, 1)],
                13: [qt_thunk(3, 2)],
                14: [qt_thunk(3, 3), proj_thunk(0, 0), proj_thunk(0, 1)],
                15: [proj_thunk(1, 0), proj_thunk(1, 1)],
            }

            pv_st = None
            pv_qc = None
            it = -1
            for p in range(PAIRS):
                for qc in range(NQC):
                    it += 1
                    kt_sb = kt_all[:, p % 2, :]
                    qt_sb = qt_all[:, p % 2, QCH * qc : QCH * (qc + 1)]
                    extras = EXTRAS.get(it, [])
                    ptl = []
                    for bi, (s0, sn) in enumerate(SLOT_BATCHES):
                        sps = psp.tile([128, sn, QCH], f32,
                                       tag=("sbA" if bi % 2 == 0 else "sbB"),
                                       bufs=1, name="sps")
                        for i in range(sn):
                            s = s0 + i
                            k, hh = s // 2, s % 2
                            sl = slice(64 * hh, 64 * (hh + 1))
                            nc.tensor.matmul(
                                sps[:, i, :],
                                kt_sb[sl, 128 * k : 128 * (k + 1)],
                                qt_sb[sl, :], start=True, stop=True)
                        ptile = ptp.tile([128, 3, QCH], bf16, name="pt")
                        ptl.append(ptile)
                        nc.scalar.activation(
                            out=ptile[:, 0:sn, :],
                            in_=sps,
                            func=EXP,
                            scale=float(SCALE),
                        )
                        if pv_st is not None:
                            emit_pv_segment(pv_st, bi)
                        if bi < len(extras):
                            extras[bi]()
                    if pv_st is not None:
                        emit_norm(pv_st, pv_qc)
                    opsl = [
                        psp.tile([HD + 1, QCH], f32, tag="ops", bufs=2,
                                 name=f"ops{hh}")
                        for hh in range(2)
                    ]
                    pv_st = (p, ptl, opsl)
                    pv_qc = qc
            # drain the last (p3, qc3); tail projs recycle the now-idle S
            # PSUM banks so they overlap the PV drain instead of queueing on
            # the single transient bank
            for seg in range(NBATCH):
                emit_pv_segment(pv_st, seg)
            emit_norm(pv_st, pv_qc)
            emit_proj_half(2, 0, tags=("sbA", "sbB"))
            emit_proj_half(2, 1, tags=("sbA", "sbB"))
            emit_proj_half(3, 0, tags=("sbA", "sbB"))
            emit_proj_half(3, 1, tags=("sbA", "sbB"))

    nc.compile()
    return nc


def get_nc():
    if "nc" not in _CACHE:
        _CACHE["nc"] = _build_nc()
    return _CACHE["nc"]


def make_in_maps(x, w_qkv, w_proj):
    import ml_dtypes

    bf = ml_dtypes.bfloat16
    in_maps = []
    for c in range(8):
        b, g = c // 2, c % 2
        in_maps.append({
            "xT": np.ascontiguousarray(x[b].T).astype(bf),
            "wqk": np.ascontiguousarray(
                np.concatenate(
                    [w_qkv[:, 512 * g : 512 * (g + 1)],
                     w_qkv[:, 1024 + 512 * g : 1024 + 512 * (g + 1)]], axis=1
                )).astype(bf),
            "wv": np.ascontiguousarray(
                w_qkv[:, 2048 + 512 * g : 2048 + 512 * (g + 1)]).astype(bf),
            "wp": np.ascontiguousarray(
                w_proj[512 * g : 512 * (g + 1), :]).astype(bf),
        })
    return in_maps


def kernel(x, w_qkv, w_proj, b_proj):
    from concourse.bass_utils import run_bass_kernel_spmd

    x = np.asarray(x, dtype=np.float32)
    w_qkv = np.asarray(w_qkv, dtype=np.float32)
    w_proj = np.asarray(w_proj, dtype=np.float32)
    b_proj = np.asarray(b_proj, dtype=np.float32)

    nc = get_nc()
    in_maps = make_in_maps(x, w_qkv, w_proj)
    res = run_bass_kernel_spmd(nc, in_maps, list(range(8))).results

    out = np.zeros((B, N, DIM), dtype=np.float32)
    for c in range(8):
        out[c // 2] += res[c]["y"]
    return out + b_proj
